# revision 10
# baseline (speedup 1.0000x reference)
"""Multi-head latent attention (MLA) Trainium2 kernel.

Sharding: 8 cores = 2 batches x 4 head-groups (4 heads each).
Per core (batch b, head-group hg):
  P1: Z = W1.T @ x_b.T, windowed over T, where
      W1 = [w_kv | w_krope | w_qrope | w_qc@w_qdec[:,hg nope cols]]
      -> c_kv^T (SBUF), rope raws -> rotate -> kqr (SBUF), q_nope^T -> DRAM qd
  P2: k_nope^T = w_kdec_hg.T @ c_kv^T -> kfull ; v = c_kv @ w_vdec_hg -> vt
  P3: per (head, q-chunk): S^T[s,q] = k_h^T.T @ q_h^T (q streamed from qd);
      causal mask on diag blocks; exp on ACT (fp32r out);
      den = DVE-accumulated exp, partition-reduced on gpsimd;
      o_h^T[d,q] = sum_s v[s,d].T @ expS^T, normalized -> DRAM od
      out[T,2048] = sum_h o_h^T.T @ w_o_h  (summed over head-group cores on host)

All matmuls float32r (tf32-like, ~11 mantissa bits, full PE rate at N>=256).
"""

import math
import numpy as np

DIM, H, D = 2048, 16, 128
D_C, D_CQ, D_ROPE = 512, 1536, 64
B, T_FULL = 2, 2048
NCORES = 8
HPG = 4            # heads per group (per core)
NCH = 64           # nope channels kept per head (rest replaced by rope)
W1N = D_C + D_ROPE + D_ROPE + HPG * NCH   # 512+64+64+256 = 896

_PROG_CACHE = {}


def _build_program(T):
    import concourse.bacc as bacc
    import concourse.tile as tile
    import concourse.mybir as mybir
    import concourse.bass_isa as bass_isa

    dt = mybir.dt
    f32, f32r = dt.float32, dt.float32r

    WIN = min(256, T)         # stage-1 T window
    nwin = T // WIN
    QB = min(512, T)          # SDPA q-chunk
    nqc = T // QB
    nr = QB // 128            # key chunks per q chunk (diag mask patterns)
    ndim = DIM // 128         # 16
    ncc = W1N // 128          # 7
    ndc = D_C // 128          # 4
    ntr = T // 128
    nsc_max = T // 128
    scale = 1.0 / math.sqrt(D)

    nc = bacc.Bacc("TRN2", target_bir_lowering=False, debug=False,
                   num_devices=NCORES)

    xt_d = nc.dram_tensor("xt", [DIM, T], f32r, kind="ExternalInput").ap()
    w1_d = nc.dram_tensor("w1", [DIM, W1N], f32r, kind="ExternalInput").ap()
    wkd_d = nc.dram_tensor("wkd", [D_C, HPG * NCH], f32r, kind="ExternalInput").ap()
    wvd_d = nc.dram_tensor("wvd", [D_C, HPG * D], f32r, kind="ExternalInput").ap()
    wo_d = nc.dram_tensor("wo", [HPG * D, DIM], f32r, kind="ExternalInput").ap()
    csc_d = nc.dram_tensor("csc", [128, T], f32, kind="ExternalInput").ap()
    css_d = nc.dram_tensor("css", [128, T], f32, kind="ExternalInput").ap()
    msk_d = nc.dram_tensor("msk", [128, nr * QB], f32, kind="ExternalInput").ap()
    out_d = nc.dram_tensor("out", [T, DIM], f32, kind="ExternalOutput").ap()

    with tile.TileContext(nc) as tc:
        with tc.tile_pool(name="pers", bufs=1) as pers, \
             tc.tile_pool(name="dram", bufs=1, space="DRAM") as dram:
            # whole-kernel persistent: 32 + 8 + 8 = 48 KB/partition
            zkv = pers.tile([128, ndc, T], f32r, tag="zkv")       # c_kv^T
            kqr = pers.tile([128, T], f32r, tag="kqr")            # [kr;qr]
            msk_t = pers.tile([128, nr, QB], f32, tag="msk")
            nc.sync.dma_start(msk_t[:], msk_d.rearrange("p (r q) -> p r q", r=nr))
            qd = dram.tile([128, HPG, T], f32r, tag="qd")         # q^T per head
            od = dram.tile([128, HPG, T], f32r, tag="od")         # o^T per head

            # ================= P1: Z projection + rope =================
            with tc.tile_pool(name="p1w", bufs=1) as p1w, \
                 tc.tile_pool(name="p1x", bufs=2) as p1x, \
                 tc.tile_pool(name="p1s", bufs=3) as p1s, \
                 tc.tile_pool(name="p1cs", bufs=2) as p1cs, \
                 tc.tile_pool(name="pp1", bufs=3, space="PSUM") as pp1:
                w1_t = p1w.tile([128, ndim, W1N], f32r, tag="w1")   # 56 KB
                nc.sync.dma_start(w1_t[:], w1_d.rearrange("(c p) n -> p c n", p=128))
                xt_r = xt_d.rearrange("(c p) t -> p c t", p=128)
                for w in range(nwin):
                    tcols = slice(w * WIN, (w + 1) * WIN)
                    xt_t = p1x.tile([128, ndim, WIN], f32r, tag="xt")  # 16 KB x2
                    nc.sync.dma_start(xt_t[:], xt_r[:, :, tcols])
                    for cc in range(ncc):
                        ps = pp1.tile([128, WIN], f32, tag="ps1")
                        for d in range(ndim):
                            nc.tensor.matmul(
                                ps[:],
                                w1_t[:, d, cc * 128:(cc + 1) * 128],
                                xt_t[:, d, :],
                                start=(d == 0), stop=(d == ndim - 1),
                            )
                        if cc < ndc:            # c_kv^T rows
                            nc.any.tensor_copy(zkv[:, cc, tcols], ps[:])
                        elif cc == ndc:         # rope raw rows -> rotate now
                            # kq[0:32]=t1*c-t2*s  kq[32:64]=t2*c+t1*s (k; same for q)
                            # css holds the sign: rows 0:32,-sin; 32:64,+sin; ...
                            csw = p1cs.tile([128, WIN], f32, tag="csw")
                            nc.sync.dma_start(csw[:], csc_d[:, tcols])
                            ssw = p1cs.tile([128, WIN], f32, tag="ssw")
                            nc.sync.dma_start(ssw[:], css_d[:, tcols])
                            rc = p1s.tile([128, WIN], f32, tag="rc")
                            nc.vector.tensor_mul(rc[:], ps[:], csw[:])
                            zraw = p1s.tile([128, WIN], f32, tag="zraw")
                            nc.any.tensor_copy(zraw[:], ps[:])
                            zrot = p1s.tile([128, WIN], f32, tag="zrot")
                            for dst, src in ((0, 32), (32, 0), (64, 96), (96, 64)):
                                nc.sync.dma_start(zrot[dst:dst + 32, :],
                                                  zraw[src:src + 32, :])
                            rrs = p1s.tile([128, WIN], f32, tag="rrs")
                            nc.vector.tensor_mul(rrs[:], zrot[:], ssw[:])
                            nc.vector.tensor_add(kqr[:, tcols], rc[:], rrs[:])
                        else:                   # q_nope^T rows -> qd nope halves
                            h0 = 2 * (cc - ndc - 1)
                            qs = p1s.tile([128, WIN], f32r, tag="qs")
                            nc.any.tensor_copy(qs[:], ps[:])
                            nc.sync.dma_start(qd[0:64, h0, tcols], qs[0:64, :])
                            nc.sync.dma_start(qd[0:64, h0 + 1, tcols], qs[64:128, :])
                # qr rows of qd (shared across heads)
                for h in range(HPG):
                    nc.sync.dma_start(qd[64:128, h, :], kqr[64:128, :])

            # ================= P2 + P3 =================
            with tc.tile_pool(name="kv", bufs=1) as kv:
                kfull = [kv.tile([128, T], f32r, tag=f"kfull{h}", name=f"kfull{h}")
                         for h in range(HPG)]                      # 32 KB
                vt = kv.tile([128, ntr, HPG * D], f32r, tag="vt")  # 32 KB

                # ---- P2: k decompress + v decompress ----
                with tc.tile_pool(name="p2w", bufs=1) as p2w, \
                     tc.tile_pool(name="pp2", bufs=3, space="PSUM") as pp2:
                    wkd_t = p2w.tile([128, ndc, HPG * NCH], f32r, tag="wkd")
                    nc.sync.dma_start(wkd_t[:], wkd_d.rearrange("(c p) n -> p c n", p=128))
                    wvd_t = p2w.tile([128, ndc, HPG * D], f32r, tag="wvd")
                    nc.sync.dma_start(wvd_t[:], wvd_d.rearrange("(c p) n -> p c n", p=128))

                    for h in range(HPG):   # rope rows of kfull (DMA: cross-partition)
                        nc.sync.dma_start(kfull[h][64:128, :], kqr[0:64, :])
                    for cc2 in range(HPG * NCH // 128):
                        for tc4 in range(max(1, T // 512)):
                            tcols = slice(tc4 * 512, min((tc4 + 1) * 512, T))
                            ncol = tcols.stop - tcols.start
                            ps = pp2.tile([128, 512], f32, tag="ps2")
                            for d in range(ndc):
                                nc.tensor.matmul(
                                    ps[:, :ncol],
                                    wkd_t[:, d, cc2 * 128:(cc2 + 1) * 128],
                                    zkv[:, d, tcols],
                                    start=(d == 0), stop=(d == ndc - 1),
                                )
                            kt = p2w.tile([128, 512], f32r, tag="kt", bufs=3)
                            nc.any.tensor_copy(kt[:, :ncol], ps[:, :ncol])
                            nc.sync.dma_start(kfull[2 * cc2][0:64, tcols], kt[0:64, :ncol])
                            nc.sync.dma_start(kfull[2 * cc2 + 1][0:64, tcols], kt[64:128, :ncol])
                    for tr in range(ntr):
                        ps = pp2.tile([128, 512], f32, tag="ps2")
                        for d in range(ndc):
                            nc.tensor.matmul(
                                ps[:],
                                zkv[:, d, tr * 128:(tr + 1) * 128],
                                wvd_t[:, d, :],
                                start=(d == 0), stop=(d == ndc - 1),
                            )
                        nc.any.tensor_copy(vt[:, tr, :], ps[:])

                # ---- P3: attention + output projection ----
                with tc.tile_pool(name="p3wo", bufs=1) as p3wo, \
                     tc.tile_pool(name="p3q", bufs=3) as p3q, \
                     tc.tile_pool(name="expp", bufs=6) as epool, \
                     tc.tile_pool(name="denp", bufs=3) as dpool, \
                     tc.tile_pool(name="p3o", bufs=3) as p3o, \
                     tc.tile_pool(name="p6i", bufs=2) as p6i, \
                     tc.tile_pool(name="outp", bufs=4) as opool, \
                     tc.tile_pool(name="pp_s", bufs=3, space="PSUM") as pp_s, \
                     tc.tile_pool(name="pp_o", bufs=2, space="PSUM") as pp_o, \
                     tc.tile_pool(name="pp_d", bufs=2, space="PSUM") as pp_d:
                    wo_t = p3wo.tile([128, HPG, DIM], f32r, tag="wo")   # 32 KB
                    nc.sync.dma_start(wo_t[:], wo_d.rearrange("(c p) n -> p c n", p=128))
                    # fp32r ones for denominator / broadcast matmuls
                    ones_f = p3o.tile([128, 128], f32, tag="ones_f", bufs=1)
                    nc.vector.memset(ones_f[:], 1.0)
                    ones_k = p3o.tile([128, 1], f32r, tag="ones_k", bufs=1)
                    nc.vector.tensor_copy(ones_k[:], ones_f[:, 0:1])
                    ones_m = p3o.tile([1, 128], f32r, tag="ones_m", bufs=1)
                    nc.vector.tensor_copy(ones_m[:], ones_f[0:1, :])

                    for h in range(HPG):
                        for qc in range(nqc):
                            qcols = slice(qc * QB, (qc + 1) * QB)
                            n_s = (qc + 1) * nr
                            qtile = p3q.tile([128, QB], f32r, tag="qtile")
                            nc.sync.dma_start(qtile[:], qd[:, h, qcols])
                            o_ps = pp_o.tile([128, QB], f32, tag="o_ps")
                            den_ps = pp_d.tile([1, QB], f32, tag="den_ps")
                            for sc in range(n_s):
                                s_ps = pp_s.tile([128, QB], f32, tag="s_ps")
                                nc.tensor.matmul(
                                    s_ps[:],
                                    kfull[h][:, sc * 128:(sc + 1) * 128],
                                    qtile[:],
                                    start=True, stop=True,
                                )
                                r = sc - qc * nr
                                if r >= 0:  # diagonal block: causal mask
                                    nc.vector.tensor_add(s_ps[:], s_ps[:], msk_t[:, r, :])
                                et = epool.tile([128, QB], f32r, tag="et")
                                nc.scalar.activation(
                                    et[:], s_ps[:], mybir.ActivationFunctionType.Exp,
                                    scale=scale,
                                )
                                # denominator: ones.T @ expS accumulated on PE
                                nc.tensor.matmul(
                                    den_ps[:], ones_k[:], et[:],
                                    start=(sc == 0), stop=(sc == n_s - 1),
                                )
                                nc.tensor.matmul(
                                    o_ps[:],
                                    vt[:, sc, h * D:(h + 1) * D],
                                    et[:],
                                    start=(sc == 0), stop=(sc == n_s - 1),
                                )
                            rec1 = dpool.tile([1, QB], f32, tag="rec1")
                            nc.vector.reciprocal(rec1[:], den_ps[:])
                            rec1r = dpool.tile([1, QB], f32r, tag="rec1r")
                            nc.any.tensor_copy(rec1r[:], rec1[:])
                            bc_ps = pp_s.tile([128, QB], f32, tag="s_ps", name="bc_ps")
                            nc.tensor.matmul(bc_ps[:], ones_m[:], rec1r[:],
                                             start=True, stop=True)
                            rec = dpool.tile([128, QB], f32, tag="rec")
                            nc.any.tensor_copy(rec[:], bc_ps[:])
                            o_sb = p3o.tile([128, QB], f32r, tag="o_sb")
                            nc.vector.tensor_mul(o_sb[:], o_ps[:], rec[:])
                            nc.sync.dma_start(od[:, h, qcols], o_sb[:])

                    # ---- stage 6: out = sum_h o_h @ w_o_h ----
                    for tr in range(ntr):
                        trcols = slice(tr * 128, (tr + 1) * 128)
                        oin = p6i.tile([128, HPG, 128], f32r, tag="oin")
                        nc.sync.dma_start(oin[:], od[:, :, trcols])
                        for oc in range(DIM // 512):
                            ps = pp_o.tile([128, 512], f32, tag="o_ps", name="ps6")
                            for h in range(HPG):
                                nc.tensor.matmul(
                                    ps[:],
                                    oin[:, h, :],
                                    wo_t[:, h, oc * 512:(oc + 1) * 512],
                                    start=(h == 0), stop=(h == HPG - 1),
                                )
                            ot = opool.tile([128, 512], f32, tag="ot")
                            nc.any.tensor_copy(ot[:], ps[:])
                            nc.sync.dma_start(
                                out_d[trcols, oc * 512:(oc + 1) * 512], ot[:],
                            )

    nc.compile()
    return nc


def _host_inputs(x, cos, sin, w_kv, w_kdec, w_vdec, w_qc, w_qdec,
                 w_krope, w_qrope, w_o, T):
    """Build the 8 per-core input maps."""
    f = np.float32
    QB = min(512, T)
    nr = QB // 128
    # masks: pattern r, element (i, j): key r*128+i vs query j
    msk = np.zeros((128, nr * QB), dtype=f)
    for r in range(nr):
        i = np.arange(128)[:, None]
        j = np.arange(QB)[None, :]
        m = np.where(r * 128 + i > j, f(-1e6), f(0.0))
        msk[:, r * QB:(r + 1) * QB] = m
    csc = np.tile(np.ascontiguousarray(cos.T), (4, 1)).astype(f)   # [128, T]
    # signs folded in: rows 0:32 -sin (pairs with swapped t2), 32:64 +sin, ...
    st = np.ascontiguousarray(sin.T)
    css = np.concatenate([-st, st, -st, st], axis=0).astype(f)

    in_maps = []
    for core in range(NCORES):
        b, hg = divmod(core, 4)
        heads = range(hg * HPG, (hg + 1) * HPG)
        qcols = np.concatenate([np.arange(h * D, h * D + NCH) for h in heads])
        w_qeff = (w_qc @ w_qdec[:, qcols]).astype(f)               # [DIM, 256]
        w1 = np.concatenate([w_kv, w_krope, w_qrope, w_qeff], axis=1).astype(f)
        wkd = np.ascontiguousarray(w_kdec[:, qcols]).astype(f)
        wvd = np.ascontiguousarray(w_vdec[:, hg * HPG * D:(hg + 1) * HPG * D]).astype(f)
        wo = np.ascontiguousarray(w_o[hg * HPG * D:(hg + 1) * HPG * D, :]).astype(f)
        xt = np.ascontiguousarray(x[b].T).astype(f)
        in_maps.append({
            "xt": xt, "w1": w1, "wkd": wkd, "wvd": wvd, "wo": wo,
            "csc": csc, "css": css, "msk": msk,
        })
    return in_maps


def _gather(results, T):
    out = np.zeros((B, T, DIM), dtype=np.float32)
    for core in range(NCORES):
        b = core // 4
        out[b] += results[core]["out"]
    return out


LAST_EXEC_NS = None


def kernel(x, cos, sin, w_kv, w_kdec, w_vdec, w_qc, w_qdec,
           w_krope, w_qrope, w_o, _trace=False):
    global LAST_EXEC_NS
    from concourse.bass_utils import run_bass_kernel_spmd

    T = x.shape[1]
    if T not in _PROG_CACHE:
        _PROG_CACHE[T] = _build_program(T)
    nc = _PROG_CACHE[T]
    in_maps = _host_inputs(x, cos, sin, w_kv, w_kdec, w_vdec, w_qc, w_qdec,
                           w_krope, w_qrope, w_o, T)
    res = run_bass_kernel_spmd(nc, in_maps, list(range(NCORES)), trace=_trace)
    LAST_EXEC_NS = res.exec_time_ns
    return _gather(res.results, T)


# revision 17
# speedup vs baseline: 1.1408x; 1.1408x over previous
"""Multi-head latent attention (MLA) Trainium2 kernel.

Sharding: 8 cores = 2 batches x 4 head-groups (4 heads each).
Per core (batch b, head-group hg):
  P1: Z = W1.T @ x_b.T, windowed over T, where
      W1 = [w_kv | w_krope | w_qrope | w_qc@w_qdec[:,hg nope cols]]
      -> c_kv^T (SBUF), rope raws -> rotate -> kqr (SBUF), q_nope^T -> DRAM qd
  P2: k_nope^T = w_kdec_hg.T @ c_kv^T -> kfull ; v = c_kv @ w_vdec_hg -> vt
  P3: per (head, q-chunk): S^T[s,q] = k_h^T.T @ q_h^T (q streamed from qd);
      causal mask on diag blocks; exp on ACT (fp32r out);
      den = DVE-accumulated exp, partition-reduced on gpsimd;
      o_h^T[d,q] = sum_s v[s,d].T @ expS^T, normalized -> DRAM od
      out[T,2048] = sum_h o_h^T.T @ w_o_h  (summed over head-group cores on host)

All matmuls float32r (tf32-like, ~11 mantissa bits, full PE rate at N>=256).
"""

import math
from contextlib import ExitStack

import numpy as np

DIM, H, D = 2048, 16, 128
D_C, D_CQ, D_ROPE = 512, 1536, 64
B, T_FULL = 2, 2048
NCORES = 8
HPG = 4            # heads per group (per core)
NCH = 64           # nope channels kept per head (rest replaced by rope)
W1N = D_C + D_ROPE + D_ROPE + HPG * NCH   # 512+64+64+256 = 896

_PROG_CACHE = {}


def _build_program(T):
    import concourse.bacc as bacc
    import concourse.tile as tile
    import concourse.mybir as mybir
    import concourse.bass_isa as bass_isa

    dt = mybir.dt
    f32, f32r = dt.float32, dt.float32r

    WIN = min(256, T)         # stage-1 T window
    nwin = T // WIN
    QB = min(512, T)          # SDPA q-chunk
    nqc = T // QB
    nr = QB // 128            # key chunks per q chunk (diag mask patterns)
    ndim = DIM // 128         # 16
    ncc = W1N // 128          # 7
    ndc = D_C // 128          # 4
    ntr = T // 128
    nsc_max = T // 128
    scale = 1.0 / math.sqrt(D)

    nc = bacc.Bacc("TRN2", target_bir_lowering=False, debug=False,
                   num_devices=NCORES)

    xt_d = nc.dram_tensor("xt", [DIM, T], f32r, kind="ExternalInput").ap()
    w1_d = nc.dram_tensor("w1", [DIM, W1N], f32r, kind="ExternalInput").ap()
    wkd_d = nc.dram_tensor("wkd", [D_C, HPG * NCH], f32r, kind="ExternalInput").ap()
    wvd_d = nc.dram_tensor("wvd", [D_C, HPG * D], f32r, kind="ExternalInput").ap()
    wo_d = nc.dram_tensor("wo", [HPG * D, DIM], f32r, kind="ExternalInput").ap()
    csc_d = nc.dram_tensor("csc", [128, T], f32, kind="ExternalInput").ap()
    css_d = nc.dram_tensor("css", [128, T], f32, kind="ExternalInput").ap()
    msk_d = nc.dram_tensor("msk", [128, nr * QB], f32, kind="ExternalInput").ap()
    out_d = nc.dram_tensor("out", [T, DIM], f32, kind="ExternalOutput").ap()

    with tile.TileContext(nc) as tc:
        with tc.tile_pool(name="pers", bufs=1) as pers, \
             tc.tile_pool(name="dram", bufs=1, space="DRAM") as dram:
            # whole-kernel persistent: kqr 8 + msk 8 + wkd 4 + wvd 8 KB/part
            kqr = pers.tile([128, T], f32r, tag="kqr")            # [kr;qr]
            msk_t = pers.tile([128, nr, QB], f32, tag="msk")
            nc.sync.dma_start(msk_t[:], msk_d.rearrange("p (r q) -> p r q", r=nr))
            qd = dram.tile([128, HPG, T], f32r, tag="qd")         # q^T per head
            od = dram.tile([128, HPG, T], f32r, tag="od")         # o^T per head
            # small weights prefetched from t=0 (used in P2)
            wkd_t = pers.tile([128, ndc, HPG * NCH], f32r, tag="wkd")
            nc.sync.dma_start(wkd_t[:], wkd_d.rearrange("(c p) n -> p c n", p=128))
            wvd_t = pers.tile([128, ndc, HPG * D], f32r, tag="wvd")
            nc.sync.dma_start(wvd_t[:], wvd_d.rearrange("(c p) n -> p c n", p=128))

            # zkv lives P1..P2 only; freed before P3 pools open
            zkv_es = ExitStack()
            zkvp = zkv_es.enter_context(tc.tile_pool(name="zkvp", bufs=1, side="right"))
            zkv = zkvp.tile([128, ndc, T], f32r, tag="zkv")       # c_kv^T 32 KB

            # ================= P1: Z projection + rope =================
            with tc.tile_pool(name="p1w", bufs=1) as p1w, \
                 tc.tile_pool(name="p1x", bufs=2) as p1x, \
                 tc.tile_pool(name="p1s", bufs=3) as p1s, \
                 tc.tile_pool(name="p1cs", bufs=2) as p1cs, \
                 tc.tile_pool(name="pp1", bufs=3, space="PSUM") as pp1:
                w1_t = p1w.tile([128, ndim, W1N], f32r, tag="w1")   # 56 KB
                w1_r = w1_d.rearrange("(c p) n -> p c n", p=128)
                for d in range(ndim):   # split so first matmuls start early
                    nc.sync.dma_start(w1_t[:, d, :], w1_r[:, d, :])
                xt_r = xt_d.rearrange("(c p) t -> p c t", p=128)
                for w in range(nwin):
                    tcols = slice(w * WIN, (w + 1) * WIN)
                    xt_t = p1x.tile([128, ndim, WIN], f32r, tag="xt")  # 16 KB x2
                    nc.sync.dma_start(xt_t[:], xt_r[:, :, tcols])
                    for cc in range(ncc):
                        ps = pp1.tile([128, WIN], f32, tag="ps1")
                        for d in range(ndim):
                            nc.tensor.matmul(
                                ps[:],
                                w1_t[:, d, cc * 128:(cc + 1) * 128],
                                xt_t[:, d, :],
                                start=(d == 0), stop=(d == ndim - 1),
                            )
                        if cc < ndc:            # c_kv^T rows
                            nc.any.tensor_copy(zkv[:, cc, tcols], ps[:])
                        elif cc == ndc:         # rope raw rows -> rotate now
                            # kq[0:32]=t1*c-t2*s  kq[32:64]=t2*c+t1*s (k; same for q)
                            # css holds the sign: rows 0:32,-sin; 32:64,+sin; ...
                            csw = p1cs.tile([128, WIN], f32, tag="csw")
                            nc.sync.dma_start(csw[:], csc_d[:, tcols])
                            ssw = p1cs.tile([128, WIN], f32, tag="ssw")
                            nc.sync.dma_start(ssw[:], css_d[:, tcols])
                            rc = p1s.tile([128, WIN], f32, tag="rc")
                            nc.vector.tensor_mul(rc[:], ps[:], csw[:])
                            zraw = p1s.tile([128, WIN], f32, tag="zraw")
                            nc.any.tensor_copy(zraw[:], ps[:])
                            zrot = p1s.tile([128, WIN], f32, tag="zrot")
                            for dst, src in ((0, 32), (32, 0), (64, 96), (96, 64)):
                                nc.sync.dma_start(zrot[dst:dst + 32, :],
                                                  zraw[src:src + 32, :])
                            rrs = p1s.tile([128, WIN], f32, tag="rrs")
                            nc.vector.tensor_mul(rrs[:], zrot[:], ssw[:])
                            nc.vector.tensor_add(kqr[:, tcols], rc[:], rrs[:])
                        else:                   # q_nope^T rows -> qd nope halves
                            h0 = 2 * (cc - ndc - 1)
                            qs = p1s.tile([128, WIN], f32r, tag="qs")
                            nc.any.tensor_copy(qs[:], ps[:])
                            nc.sync.dma_start(qd[0:64, h0, tcols], qs[0:64, :])
                            nc.sync.dma_start(qd[0:64, h0 + 1, tcols], qs[64:128, :])
                # qr rows of qd (shared across heads)
                for h in range(HPG):
                    nc.sync.dma_start(qd[64:128, h, :], kqr[64:128, :])

            # ================= P2 + P3 =================
            with tc.tile_pool(name="kv", bufs=1) as kv:
                kfull = [kv.tile([128, T], f32r, tag=f"kfull{h}", name=f"kfull{h}")
                         for h in range(HPG)]                      # 32 KB
                vt = kv.tile([128, ntr, HPG * D], f32r, tag="vt")  # 32 KB

                # ---- P2: k decompress + v decompress ----
                with tc.tile_pool(name="p2w", bufs=1) as p2w, \
                     tc.tile_pool(name="pp2", bufs=3, space="PSUM") as pp2:
                    for h in range(HPG):   # rope rows of kfull (DMA: cross-partition)
                        nc.sync.dma_start(kfull[h][64:128, :], kqr[0:64, :])
                    for cc2 in range(HPG * NCH // 128):
                        for tc4 in range(max(1, T // 512)):
                            tcols = slice(tc4 * 512, min((tc4 + 1) * 512, T))
                            ncol = tcols.stop - tcols.start
                            ps = pp2.tile([128, 512], f32, tag="ps2")
                            for d in range(ndc):
                                nc.tensor.matmul(
                                    ps[:, :ncol],
                                    wkd_t[:, d, cc2 * 128:(cc2 + 1) * 128],
                                    zkv[:, d, tcols],
                                    start=(d == 0), stop=(d == ndc - 1),
                                )
                            kt = p2w.tile([128, 512], f32r, tag="kt", bufs=3)
                            nc.any.tensor_copy(kt[:, :ncol], ps[:, :ncol])
                            nc.sync.dma_start(kfull[2 * cc2][0:64, tcols], kt[0:64, :ncol])
                            nc.sync.dma_start(kfull[2 * cc2 + 1][0:64, tcols], kt[64:128, :ncol])
                    for tr in range(ntr):
                        ps = pp2.tile([128, 512], f32, tag="ps2")
                        for d in range(ndc):
                            nc.tensor.matmul(
                                ps[:],
                                zkv[:, d, tr * 128:(tr + 1) * 128],
                                wvd_t[:, d, :],
                                start=(d == 0), stop=(d == ndc - 1),
                            )
                        nc.any.tensor_copy(vt[:, tr, :], ps[:])

                zkv_es.close()   # free zkv address space for P3 pools

                # ---- P3: attention + output projection ----
                with tc.tile_pool(name="p3wo", bufs=1) as p3wo, \
                     tc.tile_pool(name="p3q", bufs=3) as p3q, \
                     tc.tile_pool(name="expp", bufs=5) as epool, \
                     tc.tile_pool(name="denp", bufs=3) as dpool, \
                     tc.tile_pool(name="p3o", bufs=3) as p3o, \
                     tc.tile_pool(name="p6i", bufs=2) as p6i, \
                     tc.tile_pool(name="outp", bufs=3) as opool, \
                     tc.tile_pool(name="pp_s", bufs=3, space="PSUM") as pp_s, \
                     tc.tile_pool(name="pp_o", bufs=2, space="PSUM") as pp_o, \
                     tc.tile_pool(name="pp_b", bufs=1, space="PSUM") as pp_b, \
                     tc.tile_pool(name="pp_d", bufs=2, space="PSUM") as pp_d:
                    # fp32r ones for denominator / broadcast matmuls
                    ones_f = p3o.tile([128, 128], f32, tag="ones_f", bufs=1)
                    nc.vector.memset(ones_f[:], 1.0)
                    ones_k = p3o.tile([128, 1], f32r, tag="ones_k", bufs=1)
                    nc.vector.tensor_copy(ones_k[:], ones_f[:, 0:1])
                    ones_m = p3o.tile([1, 128], f32r, tag="ones_m", bufs=1)
                    nc.vector.tensor_copy(ones_m[:], ones_f[0:1, :])

                    for h in range(HPG):
                        for qc in range(nqc):
                            qcols = slice(qc * QB, (qc + 1) * QB)
                            n_s = (qc + 1) * nr
                            qtile = p3q.tile([128, QB], f32r, tag="qtile")
                            nc.sync.dma_start(qtile[:], qd[:, h, qcols])
                            o_ps = pp_o.tile([128, QB], f32, tag="o_ps")
                            den_ps = pp_d.tile([1, QB], f32, tag="den_ps")

                            def s_mm(sc):
                                s_ps = pp_s.tile([128, QB], f32, tag="s_ps",
                                                 name="s_ps")
                                nc.tensor.matmul(
                                    s_ps[:],
                                    kfull[h][:, sc * 128:(sc + 1) * 128],
                                    qtile[:],
                                    start=True, stop=True,
                                )
                                r = sc - qc * nr
                                if r >= 0:  # diagonal block: causal mask
                                    nc.vector.tensor_add(s_ps[:], s_ps[:],
                                                         msk_t[:, r, :])
                                return s_ps

                            # software pipeline: keep 2 S matmuls in flight
                            pend = [s_mm(sc) for sc in range(min(2, n_s))]
                            for sc in range(n_s):
                                if sc + 2 < n_s:
                                    pend.append(s_mm(sc + 2))
                                s_ps = pend.pop(0)
                                et = epool.tile([128, QB], f32r, tag="et")
                                nc.scalar.activation(
                                    et[:], s_ps[:], mybir.ActivationFunctionType.Exp,
                                    scale=scale,
                                )
                                # denominator: ones.T @ expS accumulated on PE
                                nc.tensor.matmul(
                                    den_ps[:], ones_k[:], et[:],
                                    start=(sc == 0), stop=(sc == n_s - 1),
                                )
                                nc.tensor.matmul(
                                    o_ps[:],
                                    vt[:, sc, h * D:(h + 1) * D],
                                    et[:],
                                    start=(sc == 0), stop=(sc == n_s - 1),
                                )
                            # broadcast den (PE) first; reciprocal off PE path
                            den_r = dpool.tile([1, QB], f32r, tag="den_r")
                            nc.any.tensor_copy(den_r[:], den_ps[:])
                            bc_ps = pp_b.tile([128, QB], f32, tag="bc_ps")
                            nc.tensor.matmul(bc_ps[:], ones_m[:], den_r[:],
                                             start=True, stop=True)
                            den_b = dpool.tile([128, QB], f32, tag="den_b")
                            nc.any.tensor_copy(den_b[:], bc_ps[:])
                            rec = dpool.tile([128, QB], f32, tag="rec")
                            nc.vector.reciprocal(rec[:], den_b[:])
                            o_sb = p3o.tile([128, QB], f32r, tag="o_sb")
                            nc.vector.tensor_mul(o_sb[:], o_ps[:], rec[:])
                            nc.sync.dma_start(od[:, h, qcols], o_sb[:])

                    # ---- stage 6: out = sum_h o_h @ w_o_h ----
                    wo_t = p3wo.tile([128, HPG, DIM], f32r, tag="wo")   # 32 KB
                    nc.sync.dma_start(wo_t[:], wo_d.rearrange("(c p) n -> p c n", p=128))
                    for tr in range(ntr):
                        trcols = slice(tr * 128, (tr + 1) * 128)
                        oin = p6i.tile([128, HPG, 128], f32r, tag="oin")
                        nc.sync.dma_start(oin[:], od[:, :, trcols])
                        for oc in range(DIM // 512):
                            ps = pp_o.tile([128, 512], f32, tag="o_ps", name="ps6")
                            for h in range(HPG):
                                nc.tensor.matmul(
                                    ps[:],
                                    oin[:, h, :],
                                    wo_t[:, h, oc * 512:(oc + 1) * 512],
                                    start=(h == 0), stop=(h == HPG - 1),
                                )
                            ot = opool.tile([128, 512], f32, tag="ot")
                            nc.any.tensor_copy(ot[:], ps[:])
                            nc.sync.dma_start(
                                out_d[trcols, oc * 512:(oc + 1) * 512], ot[:],
                            )

    nc.compile()
    return nc


def _host_inputs(x, cos, sin, w_kv, w_kdec, w_vdec, w_qc, w_qdec,
                 w_krope, w_qrope, w_o, T):
    """Build the 8 per-core input maps."""
    f = np.float32
    QB = min(512, T)
    nr = QB // 128
    # masks: pattern r, element (i, j): key r*128+i vs query j
    msk = np.zeros((128, nr * QB), dtype=f)
    for r in range(nr):
        i = np.arange(128)[:, None]
        j = np.arange(QB)[None, :]
        m = np.where(r * 128 + i > j, f(-1e6), f(0.0))
        msk[:, r * QB:(r + 1) * QB] = m
    csc = np.tile(np.ascontiguousarray(cos.T), (4, 1)).astype(f)   # [128, T]
    # signs folded in: rows 0:32 -sin (pairs with swapped t2), 32:64 +sin, ...
    st = np.ascontiguousarray(sin.T)
    css = np.concatenate([-st, st, -st, st], axis=0).astype(f)

    in_maps = []
    for core in range(NCORES):
        b, hg = divmod(core, 4)
        heads = range(hg * HPG, (hg + 1) * HPG)
        qcols = np.concatenate([np.arange(h * D, h * D + NCH) for h in heads])
        w_qeff = (w_qc @ w_qdec[:, qcols]).astype(f)               # [DIM, 256]
        w1 = np.concatenate([w_kv, w_krope, w_qrope, w_qeff], axis=1).astype(f)
        wkd = np.ascontiguousarray(w_kdec[:, qcols]).astype(f)
        wvd = np.ascontiguousarray(w_vdec[:, hg * HPG * D:(hg + 1) * HPG * D]).astype(f)
        wo = np.ascontiguousarray(w_o[hg * HPG * D:(hg + 1) * HPG * D, :]).astype(f)
        xt = np.ascontiguousarray(x[b].T).astype(f)
        in_maps.append({
            "xt": xt, "w1": w1, "wkd": wkd, "wvd": wvd, "wo": wo,
            "csc": csc, "css": css, "msk": msk,
        })
    return in_maps


def _gather(results, T):
    out = np.zeros((B, T, DIM), dtype=np.float32)
    for core in range(NCORES):
        b = core // 4
        out[b] += results[core]["out"]
    return out


LAST_EXEC_NS = None


def kernel(x, cos, sin, w_kv, w_kdec, w_vdec, w_qc, w_qdec,
           w_krope, w_qrope, w_o, _trace=False):
    global LAST_EXEC_NS
    from concourse.bass_utils import run_bass_kernel_spmd

    T = x.shape[1]
    if T not in _PROG_CACHE:
        _PROG_CACHE[T] = _build_program(T)
    nc = _PROG_CACHE[T]
    in_maps = _host_inputs(x, cos, sin, w_kv, w_kdec, w_vdec, w_qc, w_qdec,
                           w_krope, w_qrope, w_o, T)
    res = run_bass_kernel_spmd(nc, in_maps, list(range(NCORES)), trace=_trace)
    LAST_EXEC_NS = res.exec_time_ns
    return _gather(res.results, T)


# revision 18
# speedup vs baseline: 1.1675x; 1.0234x over previous
"""Multi-head latent attention (MLA) Trainium2 kernel.

Sharding: 8 cores = 2 batches x 4 head-groups (4 heads each).
Per core (batch b, head-group hg):
  P1: Z = W1.T @ x_b.T, windowed over T, where
      W1 = [w_kv | w_krope | w_qrope | w_qc@w_qdec[:,hg nope cols]]
      -> c_kv^T (SBUF), rope raws -> rotate -> kqr (SBUF), q_nope^T -> DRAM qd
  P2: k_nope^T = w_kdec_hg.T @ c_kv^T -> kfull ; v = c_kv @ w_vdec_hg -> vt
  P3: per (head, q-chunk): S^T[s,q] = k_h^T.T @ q_h^T (q streamed from qd);
      causal mask on diag blocks; exp on ACT (fp32r out);
      den = DVE-accumulated exp, partition-reduced on gpsimd;
      o_h^T[d,q] = sum_s v[s,d].T @ expS^T, normalized -> DRAM od
      out[T,2048] = sum_h o_h^T.T @ w_o_h  (summed over head-group cores on host)

All matmuls float32r (tf32-like, ~11 mantissa bits, full PE rate at N>=256).
"""

import math
from contextlib import ExitStack

import numpy as np

DIM, H, D = 2048, 16, 128
D_C, D_CQ, D_ROPE = 512, 1536, 64
B, T_FULL = 2, 2048
NCORES = 8
HPG = 4            # heads per group (per core)
NCH = 64           # nope channels kept per head (rest replaced by rope)
W1N = D_C + D_ROPE + D_ROPE + HPG * NCH   # 512+64+64+256 = 896

_PROG_CACHE = {}


def _build_program(T):
    import concourse.bacc as bacc
    import concourse.tile as tile
    import concourse.mybir as mybir
    import concourse.bass_isa as bass_isa

    dt = mybir.dt
    f32, f32r = dt.float32, dt.float32r

    WIN = min(256, T)         # stage-1 T window
    nwin = T // WIN
    QB = min(512, T)          # SDPA q-chunk
    nqc = T // QB
    nr = QB // 128            # key chunks per q chunk (diag mask patterns)
    ndim = DIM // 128         # 16
    ncc = W1N // 128          # 7
    ndc = D_C // 128          # 4
    ntr = T // 128
    nsc_max = T // 128
    scale = 1.0 / math.sqrt(D)

    nc = bacc.Bacc("TRN2", target_bir_lowering=False, debug=False,
                   num_devices=NCORES)

    xt_d = nc.dram_tensor("xt", [DIM, T], f32r, kind="ExternalInput").ap()
    w1_d = nc.dram_tensor("w1", [DIM, W1N], f32r, kind="ExternalInput").ap()
    wkd_d = nc.dram_tensor("wkd", [D_C, HPG * NCH], f32r, kind="ExternalInput").ap()
    wvd_d = nc.dram_tensor("wvd", [D_C, HPG * D], f32r, kind="ExternalInput").ap()
    wo_d = nc.dram_tensor("wo", [HPG * D, DIM], f32r, kind="ExternalInput").ap()
    csc_d = nc.dram_tensor("csc", [128, T], f32, kind="ExternalInput").ap()
    css_d = nc.dram_tensor("css", [128, T], f32, kind="ExternalInput").ap()
    msk_d = nc.dram_tensor("msk", [128, nr * QB], f32, kind="ExternalInput").ap()
    out_d = nc.dram_tensor("out", [T, DIM], f32, kind="ExternalOutput").ap()

    with tile.TileContext(nc) as tc:
        with tc.tile_pool(name="pers", bufs=1) as pers, \
             tc.tile_pool(name="dram", bufs=1, space="DRAM") as dram:
            # whole-kernel persistent: kqr 8 + msk 8 + wkd 4 + wvd 8 KB/part
            kqr = pers.tile([128, T], f32r, tag="kqr")            # [kr;qr]
            msk_t = pers.tile([128, nr, QB], f32, tag="msk")
            qd = dram.tile([128, HPG, T], f32r, tag="qd")         # q^T per head
            od = dram.tile([128, HPG, T], f32r, tag="od")         # o^T per head
            wkd_t = pers.tile([128, ndc, HPG * NCH], f32r, tag="wkd")
            wvd_t = pers.tile([128, ndc, HPG * D], f32r, tag="wvd")

            # zkv lives P1..P2 only; freed before P3 pools open
            zkv_es = ExitStack()
            zkvp = zkv_es.enter_context(tc.tile_pool(name="zkvp", bufs=1, side="right"))
            zkv = zkvp.tile([128, ndc, T], f32r, tag="zkv")       # c_kv^T 32 KB

            # ================= P1: Z projection + rope =================
            with tc.tile_pool(name="p1w", bufs=1) as p1w, \
                 tc.tile_pool(name="p1x", bufs=2) as p1x, \
                 tc.tile_pool(name="p1s", bufs=3) as p1s, \
                 tc.tile_pool(name="p1cs", bufs=2) as p1cs, \
                 tc.tile_pool(name="pp1", bufs=3, space="PSUM") as pp1:
                w1_t = p1w.tile([128, ndim, W1N], f32r, tag="w1")   # 56 KB
                w1_r = w1_d.rearrange("(c p) n -> p c n", p=128)
                xt_r = xt_d.rearrange("(c p) t -> p c t", p=128)

                def load_win(w):
                    t = p1x.tile([128, ndim, WIN], f32r, tag="xt", name="xt_t")
                    nc.sync.dma_start(t[:], xt_r[:, :, w * WIN:(w + 1) * WIN])
                    return t

                cur_x = load_win(0)
                for d in range(ndim):   # split so first matmuls start early
                    nc.sync.dma_start(w1_t[:, d, :], w1_r[:, d, :])
                for w in range(nwin):
                    tcols = slice(w * WIN, (w + 1) * WIN)
                    xt_t = cur_x
                    if w + 1 < nwin:
                        cur_x = load_win(w + 1)
                    for cc in range(ncc):
                        ps = pp1.tile([128, WIN], f32, tag="ps1")
                        for d in range(ndim):
                            nc.tensor.matmul(
                                ps[:],
                                w1_t[:, d, cc * 128:(cc + 1) * 128],
                                xt_t[:, d, :],
                                start=(d == 0), stop=(d == ndim - 1),
                            )
                        if cc < ndc:            # c_kv^T rows
                            nc.any.tensor_copy(zkv[:, cc, tcols], ps[:])
                        elif cc == ndc:         # rope raw rows -> rotate now
                            # kq[0:32]=t1*c-t2*s  kq[32:64]=t2*c+t1*s (k; same for q)
                            # css holds the sign: rows 0:32,-sin; 32:64,+sin; ...
                            csw = p1cs.tile([128, WIN], f32, tag="csw")
                            nc.sync.dma_start(csw[:], csc_d[:, tcols])
                            ssw = p1cs.tile([128, WIN], f32, tag="ssw")
                            nc.sync.dma_start(ssw[:], css_d[:, tcols])
                            rc = p1s.tile([128, WIN], f32, tag="rc")
                            nc.vector.tensor_mul(rc[:], ps[:], csw[:])
                            zraw = p1s.tile([128, WIN], f32, tag="zraw")
                            nc.any.tensor_copy(zraw[:], ps[:])
                            zrot = p1s.tile([128, WIN], f32, tag="zrot")
                            for dst, src in ((0, 32), (32, 0), (64, 96), (96, 64)):
                                nc.gpsimd.dma_start(zrot[dst:dst + 32, :],
                                                    zraw[src:src + 32, :])
                            rrs = p1s.tile([128, WIN], f32, tag="rrs")
                            nc.vector.tensor_mul(rrs[:], zrot[:], ssw[:])
                            nc.vector.tensor_add(kqr[:, tcols], rc[:], rrs[:])
                        else:                   # q_nope^T rows -> qd nope halves
                            h0 = 2 * (cc - ndc - 1)
                            qs = p1s.tile([128, WIN], f32r, tag="qs")
                            nc.any.tensor_copy(qs[:], ps[:])
                            nc.gpsimd.dma_start(qd[0:64, h0, tcols], qs[0:64, :])
                            nc.gpsimd.dma_start(qd[0:64, h0 + 1, tcols], qs[64:128, :])
                # qr rows of qd (shared across heads)
                for h in range(HPG):
                    nc.gpsimd.dma_start(qd[64:128, h, :], kqr[64:128, :])
                # P2/P3 weights + masks: load behind stage-1's input traffic
                nc.sync.dma_start(wkd_t[:], wkd_d.rearrange("(c p) n -> p c n", p=128))
                nc.sync.dma_start(wvd_t[:], wvd_d.rearrange("(c p) n -> p c n", p=128))
                nc.sync.dma_start(msk_t[:], msk_d.rearrange("p (r q) -> p r q", r=nr))

            # ================= P2 + P3 =================
            with tc.tile_pool(name="kv", bufs=1) as kv:
                kfull = [kv.tile([128, T], f32r, tag=f"kfull{h}", name=f"kfull{h}")
                         for h in range(HPG)]                      # 32 KB
                vt = kv.tile([128, ntr, HPG * D], f32r, tag="vt")  # 32 KB

                # ---- P2: k decompress + v decompress ----
                with tc.tile_pool(name="p2w", bufs=1) as p2w, \
                     tc.tile_pool(name="pp2", bufs=3, space="PSUM") as pp2:
                    for h in range(HPG):   # rope rows of kfull (DMA: cross-partition)
                        nc.gpsimd.dma_start(kfull[h][64:128, :], kqr[0:64, :])
                    for cc2 in range(HPG * NCH // 128):
                        for tc4 in range(max(1, T // 512)):
                            tcols = slice(tc4 * 512, min((tc4 + 1) * 512, T))
                            ncol = tcols.stop - tcols.start
                            ps = pp2.tile([128, 512], f32, tag="ps2")
                            for d in range(ndc):
                                nc.tensor.matmul(
                                    ps[:, :ncol],
                                    wkd_t[:, d, cc2 * 128:(cc2 + 1) * 128],
                                    zkv[:, d, tcols],
                                    start=(d == 0), stop=(d == ndc - 1),
                                )
                            kt = p2w.tile([128, 512], f32r, tag="kt", bufs=3)
                            nc.any.tensor_copy(kt[:, :ncol], ps[:, :ncol])
                            nc.gpsimd.dma_start(kfull[2 * cc2][0:64, tcols], kt[0:64, :ncol])
                            nc.gpsimd.dma_start(kfull[2 * cc2 + 1][0:64, tcols], kt[64:128, :ncol])
                    for tr in range(ntr):
                        ps = pp2.tile([128, 512], f32, tag="ps2")
                        for d in range(ndc):
                            nc.tensor.matmul(
                                ps[:],
                                zkv[:, d, tr * 128:(tr + 1) * 128],
                                wvd_t[:, d, :],
                                start=(d == 0), stop=(d == ndc - 1),
                            )
                        nc.any.tensor_copy(vt[:, tr, :], ps[:])

                zkv_es.close()   # free zkv address space for P3 pools

                # ---- P3: attention + output projection ----
                with tc.tile_pool(name="p3wo", bufs=1) as p3wo, \
                     tc.tile_pool(name="p3q", bufs=3) as p3q, \
                     tc.tile_pool(name="expp", bufs=5) as epool, \
                     tc.tile_pool(name="denp", bufs=3) as dpool, \
                     tc.tile_pool(name="p3o", bufs=3) as p3o, \
                     tc.tile_pool(name="p6i", bufs=2) as p6i, \
                     tc.tile_pool(name="outp", bufs=2) as opool, \
                     tc.tile_pool(name="pp_s", bufs=3, space="PSUM") as pp_s, \
                     tc.tile_pool(name="pp_o", bufs=3, space="PSUM") as pp_o, \
                     tc.tile_pool(name="pp_b", bufs=1, space="PSUM") as pp_b, \
                     tc.tile_pool(name="pp_d", bufs=1, space="PSUM") as pp_d:
                    # fp32r ones for denominator / broadcast matmuls
                    ones_f = p3o.tile([128, 128], f32, tag="ones_f", bufs=1)
                    nc.vector.memset(ones_f[:], 1.0)
                    ones_k = p3o.tile([128, 1], f32r, tag="ones_k", bufs=1)
                    nc.vector.tensor_copy(ones_k[:], ones_f[:, 0:1])
                    ones_m = p3o.tile([1, 128], f32r, tag="ones_m", bufs=1)
                    nc.vector.tensor_copy(ones_m[:], ones_f[0:1, :])

                    for h in range(HPG):
                        for qc in range(nqc):
                            qcols = slice(qc * QB, (qc + 1) * QB)
                            n_s = (qc + 1) * nr
                            qtile = p3q.tile([128, QB], f32r, tag="qtile")
                            nc.gpsimd.dma_start(qtile[:], qd[:, h, qcols])
                            o_ps = pp_o.tile([128, QB], f32, tag="o_ps")
                            den_ps = pp_d.tile([1, QB], f32, tag="den_ps")

                            def s_mm(sc):
                                s_ps = pp_s.tile([128, QB], f32, tag="s_ps",
                                                 name="s_ps")
                                nc.tensor.matmul(
                                    s_ps[:],
                                    kfull[h][:, sc * 128:(sc + 1) * 128],
                                    qtile[:],
                                    start=True, stop=True,
                                )
                                r = sc - qc * nr
                                if r >= 0:  # diagonal block: causal mask
                                    nc.vector.tensor_add(s_ps[:], s_ps[:],
                                                         msk_t[:, r, :])
                                return s_ps

                            # software pipeline: keep 2 S matmuls in flight
                            pend = [s_mm(sc) for sc in range(min(2, n_s))]
                            for sc in range(n_s):
                                if sc + 2 < n_s:
                                    pend.append(s_mm(sc + 2))
                                s_ps = pend.pop(0)
                                et = epool.tile([128, QB], f32r, tag="et")
                                nc.scalar.activation(
                                    et[:], s_ps[:], mybir.ActivationFunctionType.Exp,
                                    scale=scale,
                                )
                                # denominator: ones.T @ expS accumulated on PE
                                nc.tensor.matmul(
                                    den_ps[:], ones_k[:], et[:],
                                    start=(sc == 0), stop=(sc == n_s - 1),
                                )
                                nc.tensor.matmul(
                                    o_ps[:],
                                    vt[:, sc, h * D:(h + 1) * D],
                                    et[:],
                                    start=(sc == 0), stop=(sc == n_s - 1),
                                )
                            # broadcast den (PE) first; reciprocal off PE path
                            den_r = dpool.tile([1, QB], f32r, tag="den_r")
                            nc.any.tensor_copy(den_r[:], den_ps[:])
                            bc_ps = pp_b.tile([128, QB], f32, tag="bc_ps")
                            nc.tensor.matmul(bc_ps[:], ones_m[:], den_r[:],
                                             start=True, stop=True)
                            den_b = dpool.tile([128, QB], f32, tag="den_b")
                            nc.any.tensor_copy(den_b[:], bc_ps[:])
                            rec = dpool.tile([128, QB], f32, tag="rec")
                            nc.vector.reciprocal(rec[:], den_b[:])
                            o_sb = p3o.tile([128, QB], f32r, tag="o_sb")
                            nc.vector.tensor_mul(o_sb[:], o_ps[:], rec[:])
                            nc.gpsimd.dma_start(od[:, h, qcols], o_sb[:])

                    # ---- stage 6: out = sum_h o_h @ w_o_h ----
                    wo_t = p3wo.tile([128, HPG, DIM], f32r, tag="wo")   # 32 KB
                    nc.sync.dma_start(wo_t[:], wo_d.rearrange("(c p) n -> p c n", p=128))
                    for tr in range(ntr):
                        trcols = slice(tr * 128, (tr + 1) * 128)
                        oin = p6i.tile([128, HPG, 128], f32r, tag="oin")
                        nc.gpsimd.dma_start(oin[:], od[:, :, trcols])
                        orow = opool.tile([128, DIM], f32, tag="orow")
                        for oc in range(DIM // 512):
                            ps = pp_o.tile([128, 512], f32, tag="o_ps", name="ps6")
                            for h in range(HPG):
                                nc.tensor.matmul(
                                    ps[:],
                                    oin[:, h, :],
                                    wo_t[:, h, oc * 512:(oc + 1) * 512],
                                    start=(h == 0), stop=(h == HPG - 1),
                                )
                            nc.any.tensor_copy(orow[:, oc * 512:(oc + 1) * 512], ps[:])
                        nc.sync.dma_start(out_d[trcols, :], orow[:])

    nc.compile()
    return nc


def _host_inputs(x, cos, sin, w_kv, w_kdec, w_vdec, w_qc, w_qdec,
                 w_krope, w_qrope, w_o, T):
    """Build the 8 per-core input maps."""
    f = np.float32
    QB = min(512, T)
    nr = QB // 128
    # masks: pattern r, element (i, j): key r*128+i vs query j
    msk = np.zeros((128, nr * QB), dtype=f)
    for r in range(nr):
        i = np.arange(128)[:, None]
        j = np.arange(QB)[None, :]
        m = np.where(r * 128 + i > j, f(-1e6), f(0.0))
        msk[:, r * QB:(r + 1) * QB] = m
    csc = np.tile(np.ascontiguousarray(cos.T), (4, 1)).astype(f)   # [128, T]
    # signs folded in: rows 0:32 -sin (pairs with swapped t2), 32:64 +sin, ...
    st = np.ascontiguousarray(sin.T)
    css = np.concatenate([-st, st, -st, st], axis=0).astype(f)

    in_maps = []
    for core in range(NCORES):
        b, hg = divmod(core, 4)
        heads = range(hg * HPG, (hg + 1) * HPG)
        qcols = np.concatenate([np.arange(h * D, h * D + NCH) for h in heads])
        w_qeff = (w_qc @ w_qdec[:, qcols]).astype(f)               # [DIM, 256]
        w1 = np.concatenate([w_kv, w_krope, w_qrope, w_qeff], axis=1).astype(f)
        wkd = np.ascontiguousarray(w_kdec[:, qcols]).astype(f)
        wvd = np.ascontiguousarray(w_vdec[:, hg * HPG * D:(hg + 1) * HPG * D]).astype(f)
        wo = np.ascontiguousarray(w_o[hg * HPG * D:(hg + 1) * HPG * D, :]).astype(f)
        xt = np.ascontiguousarray(x[b].T).astype(f)
        in_maps.append({
            "xt": xt, "w1": w1, "wkd": wkd, "wvd": wvd, "wo": wo,
            "csc": csc, "css": css, "msk": msk,
        })
    return in_maps


def _gather(results, T):
    out = np.zeros((B, T, DIM), dtype=np.float32)
    for core in range(NCORES):
        b = core // 4
        out[b] += results[core]["out"]
    return out


LAST_EXEC_NS = None


def kernel(x, cos, sin, w_kv, w_kdec, w_vdec, w_qc, w_qdec,
           w_krope, w_qrope, w_o, _trace=False):
    global LAST_EXEC_NS
    from concourse.bass_utils import run_bass_kernel_spmd

    T = x.shape[1]
    if T not in _PROG_CACHE:
        _PROG_CACHE[T] = _build_program(T)
    nc = _PROG_CACHE[T]
    in_maps = _host_inputs(x, cos, sin, w_kv, w_kdec, w_vdec, w_qc, w_qdec,
                           w_krope, w_qrope, w_o, T)
    res = run_bass_kernel_spmd(nc, in_maps, list(range(NCORES)), trace=_trace)
    LAST_EXEC_NS = res.exec_time_ns
    return _gather(res.results, T)
